# revision 12
# baseline (speedup 1.0000x reference)
"""Multi-head attention (b=4, n=2048, dim=1024, heads=16) on 8 TRN2 cores.

Sharding: tensor-parallel over heads (2 heads per core) + row-parallel output
projection; host sums the 8 partial outputs and adds the bias.

Schedule: the TRN2 PE drops to a half-speed p-state after any idle gap and
needs 3us of continuous execution to re-ramp, so the kernel keeps the PE
busy end-to-end: attention (scores -> exp -> attnV) is locally exp-bound on
the scalar engine, and the spare PE cycles are filled by interleaving the
next batch's QKV projection matmuls and deferred output-projection matmuls
as rationed filler (plus throwaway dummy matmuls when real filler runs dry).

Per-core math (heads h0=2c, h1=2c+1), one chunk = (batch, query-half, head):
  qkv^T = w_in_c^T @ x^T            (filler, f32 psum, 8-step k chains)
  S^T   = k_h^T.T @ q_h^T           (bf16, K=64, f32 psum)
  E^T   = exp(S^T / 8)              (ACT, no max subtraction: scores ~ N(0,1))
  [o^T; denom] = [v_h | 1].T @ E^T  (bf16, M=65 fuses softmax denominator)
  o_norm^T = o^T * (1/denom)        (DVE recip on [128,16] + DMA broadcast)
  partial^T = w_out_c.T @ o_norm^T  (filler) -> DRAM bf16
"""

import os
import sys
import types

import numpy as np

# NTFF-profile hook shim: container's antenv lacks axon_hooks; harmless if
# tracing is never requested.
if "antenv.axon_hooks" not in sys.modules:
    try:
        from trn_agent_boot.trn_boot import _ntff_profile_via_ctypes

        _m = types.ModuleType("antenv.axon_hooks")
        _h = _ntff_profile_via_ctypes("/opt/axon/libaxon_pjrt.so")
        _m.get_axon_ntff_profile_hook = lambda: _h
        _m.set_axon_ntff_profile_hook = lambda hook: None
        sys.modules["antenv.axon_hooks"] = _m
    except Exception:
        pass

import ml_dtypes

import concourse.bacc as bacc
import concourse.bass as bass
import concourse.mybir as mybir
import concourse.tile as tile
from concourse.bass_utils import run_bass_kernel_spmd
from concourse.masks import make_identity

F32 = mybir.dt.float32
BF16 = mybir.dt.bfloat16

B, N, DIM, HEADS = 4, 2048, 1024, 16
HD = DIM // HEADS          # 64
NCORES = 8
HPC = HEADS // NCORES      # 2 heads per core
NT = B * N                 # 8192 tokens
MQKV = 3 * HPC * HD        # 384 qkv output dims per core
SCALE = HD ** -0.5         # 0.125

KT_TILES = DIM // 128      # 8 k-tiles in the projection contraction
JT = N // 128              # 16 j-tiles per batch
NNB = NT // 1024           # 8 token-blocks for qkv

USE_DMA_TRANSPOSE = False


def _build_nc():
    nc = bacc.Bacc("TRN2", target_bir_lowering=False, debug=False)

    xT = nc.dram_tensor("xT", [DIM, NT], BF16, kind="ExternalInput")
    w_in_c = nc.dram_tensor("w_in_c", [DIM, MQKV], BF16, kind="ExternalInput")
    w_out_c = nc.dram_tensor("w_out_c", [128, DIM], BF16, kind="ExternalInput")
    po = nc.dram_tensor("po", [DIM, NT], BF16, kind="ExternalOutput")
    dn_dram = nc.dram_tensor("dn_dram", [16, 1024], F32)
    rc_dram = nc.dram_tensor("rc_dram", [16, 1024], BF16)

    xT_r = xT.rearrange("(kt p) n -> p kt n", p=128)

    with tile.TileContext(nc) as tc:
        with (
            tc.tile_pool(name="big", bufs=1) as big,
            tc.tile_pool(name="xinp", bufs=3) as xinp,
            tc.tile_pool(name="strm", bufs=2) as strm,
            tc.tile_pool(name="et", bufs=6) as etp,
            tc.tile_pool(name="pout", bufs=4) as poutp,
            tc.tile_pool(name="stp", bufs=2, space="PSUM") as stp,
            tc.tile_pool(name="pop", bufs=1, space="PSUM") as popp,
            tc.tile_pool(name="fillp", bufs=2, space="PSUM") as fillp,
        ):
            # ---- persistent SBUF ----
            QT = big.tile([128, NT], BF16)    # [q_h0(0:64); q_h1(64:128)]^T
            KT = big.tile([128, NT], BF16)
            Vt = big.tile([128, B * JT, 130], BF16)  # [v_h0|1|v_h1|1] per j-tile
            o_sb = big.tile([128, NT], BF16)  # o^T both heads (normed in place)
            w_in_sb = big.tile([128, KT_TILES, MQKV], BF16)
            w_out_sb = big.tile([128, DIM], BF16)

            nc.sync.dma_start(
                out=w_in_sb,
                in_=w_in_c.rearrange("(kt p) m -> p kt m", p=128),
            )
            nc.sync.dma_start(out=w_out_sb, in_=w_out_c[:, :])
            if not USE_DMA_TRANSPOSE:
                ident = big.tile([128, 128], BF16)
                make_identity(nc, ident)
            nc.vector.memset(Vt[:, :, 64], 1.0)
            nc.vector.memset(Vt[:, :, 129], 1.0)

            xin_tiles = {}

            def emit_xin_dma(nb):
                xin = xinp.tile(
                    [128, KT_TILES, 2, 512], BF16, tag="xin", name=f"xin{nb}"
                )
                ncol = slice(nb * 1024, (nb + 1) * 1024)
                src = xT_r[:, :, ncol].rearrange("p kt (a b) -> p kt a b", b=512)
                for k in range(KT_TILES):
                    for a in range(2):
                        nc.sync.dma_start(
                            out=xin[:, k, a, :], in_=src[:, k, a, :]
                        )
                xin_tiles[nb] = xin

            # ---------- filler unit generators ----------
            def qkv_unit(nb, m, a):
                """8-chain projection matmul unit -> QT/KT/Vt columns."""
                pj = fillp.tile([128, 512], F32, tag="fill", name=f"pj{nb}{m}{a}")
                xin = xin_tiles[nb]
                for k in range(KT_TILES):
                    nc.tensor.matmul(
                        pj,
                        w_in_sb[:, k, m * 128:(m + 1) * 128],
                        xin[:, k, a, :],
                        start=(k == 0),
                        stop=(k == KT_TILES - 1),
                    )
                    yield 1
                cols = slice(nb * 1024 + a * 512, nb * 1024 + (a + 1) * 512)
                if m == 0:
                    nc.vector.tensor_copy(QT[:, cols], pj)
                elif m == 1:
                    nc.vector.tensor_copy(KT[:, cols], pj)
                else:
                    vstage = strm.tile([128, 512], BF16, tag="vstage")
                    nc.vector.tensor_copy(vstage, pj)
                    g0 = nb * 8 + a * 4
                    if USE_DMA_TRANSPOSE:
                        nc.sync.dma_start_transpose(
                            out=Vt[:, g0:g0 + 4, 0:64], in_=vstage[0:64, :]
                        )
                        nc.sync.dma_start_transpose(
                            out=Vt[:, g0:g0 + 4, 65:129], in_=vstage[64:128, :]
                        )
                        yield 0
                    else:
                        for c in range(4):
                            tp = fillp.tile(
                                [128, 128], BF16, tag="fill", name=f"tp{g0 + c}"
                            )
                            nc.tensor.transpose(
                                tp, vstage[:, c * 128:(c + 1) * 128], ident
                            )
                            # transpose LDWEIGHTS (~176ns) exceeds its compute
                            # (~92ns): cost 2 so consecutive transposes land in
                            # different slots with 512-wide matmuls between
                            yield 2
                            nc.vector.tensor_copy(Vt[:, g0 + c, 0:64], tp[:, 0:64])
                            nc.vector.tensor_copy(
                                Vt[:, g0 + c, 65:129], tp[:, 64:128]
                            )
                        yield 0

            def proj_unit(bp, ihp, mt, a):
                """One output-projection matmul -> po DRAM (bf16 partial)."""
                i0 = bp * N + ihp * 1024 + a * 512
                pp = fillp.tile(
                    [128, 512], F32, tag="fill", name=f"pp{bp}{ihp}{mt}{a}"
                )
                nc.tensor.matmul(
                    pp,
                    w_out_sb[:, mt * 128:(mt + 1) * 128],
                    o_sb[:, i0:i0 + 512],
                    start=True,
                    stop=True,
                )
                yield 1
                pb = poutp.tile([128, 512], BF16, tag="pout")
                nc.vector.tensor_copy(pb, pp)
                nc.sync.dma_start(
                    out=po[mt * 128:(mt + 1) * 128, i0:i0 + 512], in_=pb
                )
                yield 0

            def dma_unit(fn, *args):
                fn(*args)
                yield 0

            class Filler:
                def __init__(self):
                    self.q = []
                    self.ndummy = 0

                def add(self, gen):
                    self.q.append(gen)

                def pop(self, n, dummy_ok=True):
                    got = 0
                    while got < n:
                        if not self.q:
                            if not dummy_ok:
                                return got
                            # dummy matmul: keeps the PE p-state ramped when
                            # real filler is exhausted; result never read
                            dj = fillp.tile(
                                [128, 512], F32, tag="fill",
                                name=f"dj{self.ndummy}",
                            )
                            self.ndummy += 1
                            nc.tensor.matmul(
                                dj, w_out_sb[:, 0:128], w_out_sb[:, 0:512],
                                start=True, stop=True,
                            )
                            got += 1
                            continue
                        try:
                            got += next(self.q[0])
                        except StopIteration:
                            self.q.pop(0)
                    return got

            filler = Filler()

            # ---------- attention chunk machinery ----------
            chunks = [
                (b, ih, h)
                for b in range(B)
                for ih in range(2)
                for h in range(HPC)
            ]

            def emit_scores(b, ih, h, jt, ci):
                hp = slice(h * 64, (h + 1) * 64)
                i0 = b * N + ih * 1024
                jcol = slice(b * N + jt * 128, b * N + jt * 128 + 128)
                st = stp.tile(
                    [128, 2, 512], F32, tag="st", name=f"st{ci}_{jt}"
                )
                for a in range(2):
                    nc.tensor.matmul(
                        st[:, a, :],
                        KT[hp, jcol],
                        QT[hp, i0 + a * 512:i0 + (a + 1) * 512],
                        start=True,
                        stop=True,
                    )
                et = etp.tile([128, 1024], BF16, tag="et", name="et")
                nc.scalar.activation(
                    et,
                    st.rearrange("p a b -> p (a b)"),
                    mybir.ActivationFunctionType.Exp,
                    scale=SCALE,
                )
                return et

            def emit_attnv(b, h, jp, et, po_t):
                for a in range(2):
                    nc.tensor.matmul(
                        po_t[:, a, :],
                        Vt[:, b * JT + jp, h * 65:h * 65 + 65],
                        et[:, a * 512:(a + 1) * 512],
                        start=(jp == 0),
                        stop=(jp == JT - 1),
                    )

            def emit_drain(b, ih, h, po_t):
                i0 = b * N + ih * 1024
                icol = slice(i0, i0 + 1024)
                seg = b * 4 + ih * 2 + h
                po_f = po_t.rearrange("p a b -> p (a b)")
                if h == 0:
                    nc.vector.tensor_copy(o_sb[0:64, icol], po_f[0:64, :])
                else:
                    h1s = strm.tile([64, 1024], BF16, tag="h1s")
                    nc.vector.tensor_copy(h1s, po_f[0:64, :])
                    nc.sync.dma_start(out=o_sb[64:128, icol], in_=h1s)
                dnrow = strm.tile([1, 1024], F32, tag="dnrow")
                nc.vector.tensor_copy(dnrow, po_f[64:65, :])
                nc.sync.dma_start(out=dn_dram[seg:seg + 1, :], in_=dnrow)

            def emit_normalize(b, ih, hh):
                # per-head normalize, emitted one chunk after the drain so the
                # dn DRAM round trip completes before the DVE touches it
                i0 = b * N + ih * 1024
                icol = slice(i0, i0 + 1024)
                seg = b * 4 + ih * 2 + hh
                dns = strm.tile([128, 8], F32, tag="dns")
                nc.sync.dma_start(
                    out=dns,
                    in_=dn_dram[seg:seg + 1, :].rearrange(
                        "o (p a) -> (o p) a", p=128
                    ),
                )
                with nc.allow_low_precision(reason="softmax denom recip"):
                    nc.vector.reciprocal(dns, dns)
                rcc = strm.tile([128, 8], BF16, tag="rcc")
                nc.vector.tensor_copy(rcc, dns)
                nc.sync.dma_start(
                    out=rc_dram[seg:seg + 1, :].rearrange(
                        "o (p a) -> (o p) a", p=128
                    ),
                    in_=rcc,
                )
                rows = slice(hh * 64, (hh + 1) * 64)
                bcast = strm.tile([128, 1024], BF16, tag="bcast")
                src = rc_dram[seg:seg + 1, :]
                rbc = bass.AP(
                    tensor=src.tensor,
                    offset=src.offset,
                    ap=[[0, 64]] + list(src.ap)[1:],
                )
                nc.sync.dma_start(out=bcast[rows, :], in_=rbc)
                nc.vector.tensor_mul(
                    o_sb[rows, icol], o_sb[rows, icol], bcast[rows, :]
                )

            # ---------- build filler queue: prologue qkv for b0 ----------
            emit_xin_dma(0)
            emit_xin_dma(1)
            for nb in range(2):
                for m in range(3):
                    for a in range(2):
                        filler.add(qkv_unit(nb, m, a))
            filler.pop(10 ** 6, dummy_ok=False)  # prologue: drain all of b0

            # queue qkv for batches 1..3 (consumed as filler during
            # attention); keep xin DMAs 2 blocks ahead of their consumers
            emit_xin_dma(2)
            emit_xin_dma(3)
            for nb in range(2, NNB):
                if nb + 2 < NNB:
                    filler.add(dma_unit(emit_xin_dma, nb + 2))
                for m in range(3):
                    for a in range(2):
                        filler.add(qkv_unit(nb, m, a))

            # ---------- main loop ----------
            pending_norm = []
            for ci, (b, ih, h) in enumerate(chunks):
                ets = {}
                po_t = popp.tile(
                    [65, 2, 512], F32, tag="po", name=f"po{ci}"
                )
                # per-slot tensor surplus over the ~1.1us exp cadence keeps
                # the PE off the st-buffer dependency wall (max p-state);
                # dummies top up when the real queue runs dry
                rate = 2.0
                acc = 3.0  # chunk-start boost: covers prev drain latency
                for jt in range(JT):
                    ets[jt] = emit_scores(b, ih, h, jt, ci)
                    if jt >= 2:
                        emit_attnv(b, h, jt - 2, ets.pop(jt - 2), po_t)
                    if jt == 2:
                        while pending_norm:
                            nb_, nih_, nh_ = pending_norm.pop(0)
                            emit_normalize(nb_, nih_, nh_)
                            if nh_ == 1:
                                for mt in range(DIM // 128):
                                    for a in range(2):
                                        filler.add(
                                            proj_unit(nb_, nih_, mt, a)
                                        )
                    acc += rate
                    npop = int(acc)
                    if npop:
                        filler.pop(npop)
                        acc -= npop
                emit_attnv(b, h, JT - 2, ets.pop(JT - 2), po_t)
                filler.pop(1)
                emit_attnv(b, h, JT - 1, ets.pop(JT - 1), po_t)
                emit_drain(b, ih, h, po_t)
                pending_norm.append((b, ih, h))
                filler.pop(2)

            # epilogue: last chunk's normalize + projection; dummy matmuls
            # keep the PE ramped while the dn/rc DMA round trips complete
            filler.pop(10)
            while pending_norm:
                nb_, nih_, nh_ = pending_norm.pop(0)
                emit_normalize(nb_, nih_, nh_)
                if nh_ == 1:
                    for mt in range(DIM // 128):
                        for a in range(2):
                            filler.add(proj_unit(nb_, nih_, mt, a))
            filler.pop(25)
            filler.pop(10 ** 6, dummy_ok=False)

    nc.finalize()
    return nc


_CACHED = {}


def kernel(x, w_in, w_out, b_out, _trace=False):
    if "nc" not in _CACHED:
        _CACHED["nc"] = _build_nc()
    nc = _CACHED["nc"]

    x2 = np.ascontiguousarray(
        x.reshape(NT, DIM).T.astype(np.float32)
    )  # [DIM, NT]
    in_maps = []
    for c in range(NCORES):
        h0, h1 = HPC * c, HPC * c + 1
        cols = []
        for part in range(3):  # q, k, v
            base = part * DIM
            cols.extend(range(base + h0 * HD, base + h0 * HD + HD))
            cols.extend(range(base + h1 * HD, base + h1 * HD + HD))
        w_in_cc = np.ascontiguousarray(w_in[:, cols].astype(np.float32))
        w_out_cc = np.ascontiguousarray(
            w_out[128 * c:128 * (c + 1), :].astype(np.float32)
        )
        in_maps.append(
            {
                "xT": x2.astype(ml_dtypes.bfloat16),
                "w_in_c": w_in_cc.astype(ml_dtypes.bfloat16),
                "w_out_c": w_out_cc.astype(ml_dtypes.bfloat16),
            }
        )

    res = run_bass_kernel_spmd(
        nc, in_maps, core_ids=list(range(NCORES)), trace=_trace
    )
    acc = res.results[0]["po"].astype(np.float64)
    for c in range(1, NCORES):
        acc = acc + res.results[c]["po"].astype(np.float64)
    out = acc.T + b_out.astype(np.float64)
    if _trace:
        kernel.last_result = res
    return np.ascontiguousarray(out.reshape(B, N, DIM).astype(np.float32))


# revision 14
# speedup vs baseline: 1.0321x; 1.0321x over previous
"""Multi-head attention (b=4, n=2048, dim=1024, heads=16) on 8 TRN2 cores.

Sharding: tensor-parallel over heads (2 heads per core) + row-parallel output
projection; host sums the 8 partial outputs and adds the bias.

Schedule: the TRN2 PE drops to a half-speed p-state after any idle gap and
needs 3us of continuous execution to re-ramp, so the kernel keeps the PE
busy end-to-end: attention (scores -> exp -> attnV) is locally exp-bound on
the scalar engine, and the spare PE cycles are filled by interleaving the
next batch's QKV projection matmuls and deferred output-projection matmuls
as rationed filler (plus throwaway dummy matmuls when real filler runs dry).

Per-core math (heads h0=2c, h1=2c+1), one chunk = (batch, query-half, head):
  qkv^T = w_in_c^T @ x^T            (filler, f32 psum, 8-step k chains)
  S^T   = k_h^T.T @ q_h^T           (bf16, K=64, f32 psum)
  E^T   = exp(S^T / 8)              (ACT, no max subtraction: scores ~ N(0,1))
  [o^T; denom] = [v_h | 1].T @ E^T  (bf16, M=65 fuses softmax denominator)
  o_norm^T = o^T * (1/denom)        (DVE recip on [128,16] + DMA broadcast)
  partial^T = w_out_c.T @ o_norm^T  (filler) -> DRAM bf16
"""

import os
import sys
import types

import numpy as np

# NTFF-profile hook shim: container's antenv lacks axon_hooks; harmless if
# tracing is never requested.
if "antenv.axon_hooks" not in sys.modules:
    try:
        from trn_agent_boot.trn_boot import _ntff_profile_via_ctypes

        _m = types.ModuleType("antenv.axon_hooks")
        _h = _ntff_profile_via_ctypes("/opt/axon/libaxon_pjrt.so")
        _m.get_axon_ntff_profile_hook = lambda: _h
        _m.set_axon_ntff_profile_hook = lambda hook: None
        sys.modules["antenv.axon_hooks"] = _m
    except Exception:
        pass

import ml_dtypes

import concourse.bacc as bacc
import concourse.bass as bass
import concourse.mybir as mybir
import concourse.tile as tile
from concourse.bass_utils import run_bass_kernel_spmd
from concourse.masks import make_identity

F32 = mybir.dt.float32
BF16 = mybir.dt.bfloat16

B, N, DIM, HEADS = 4, 2048, 1024, 16
HD = DIM // HEADS          # 64
NCORES = 8
HPC = HEADS // NCORES      # 2 heads per core
NT = B * N                 # 8192 tokens
MQKV = 3 * HPC * HD        # 384 qkv output dims per core
SCALE = HD ** -0.5         # 0.125

KT_TILES = DIM // 128      # 8 k-tiles in the projection contraction
JT = N // 128              # 16 j-tiles per batch
NNB = NT // 1024           # 8 token-blocks for qkv

USE_DMA_TRANSPOSE = False


def _build_nc():
    nc = bacc.Bacc("TRN2", target_bir_lowering=False, debug=False)

    xT = nc.dram_tensor("xT", [DIM, NT], BF16, kind="ExternalInput")
    w_in_c = nc.dram_tensor("w_in_c", [DIM, MQKV], BF16, kind="ExternalInput")
    w_out_c = nc.dram_tensor("w_out_c", [128, DIM], BF16, kind="ExternalInput")
    po = nc.dram_tensor("po", [DIM, NT], BF16, kind="ExternalOutput")
    dn_dram = nc.dram_tensor("dn_dram", [16, 1024], BF16)
    rc_dram = nc.dram_tensor("rc_dram", [16, 1024], BF16)

    xT_r = xT.rearrange("(kt p) n -> p kt n", p=128)

    with tile.TileContext(nc) as tc:
        with (
            tc.tile_pool(name="big", bufs=1) as big,
            tc.tile_pool(name="xinp", bufs=3) as xinp,
            tc.tile_pool(name="strm", bufs=2) as strm,
            tc.tile_pool(name="et", bufs=6) as etp,
            tc.tile_pool(name="pout", bufs=4) as poutp,
            tc.tile_pool(name="stp", bufs=2, space="PSUM") as stp,
            tc.tile_pool(name="pop", bufs=1, space="PSUM") as popp,
            tc.tile_pool(name="fillp", bufs=2, space="PSUM") as fillp,
        ):
            # ---- persistent SBUF ----
            QT = big.tile([128, NT], BF16)    # [q_h0(0:64); q_h1(64:128)]^T
            KT = big.tile([128, NT], BF16)
            Vt = big.tile([128, B * JT, 130], BF16)  # [v_h0|1|v_h1|1] per j-tile
            o_sb = big.tile([128, NT], BF16)  # o^T both heads (normed in place)
            w_in_sb = big.tile([128, KT_TILES, MQKV], BF16)
            w_out_sb = big.tile([128, DIM], BF16)

            nc.sync.dma_start(
                out=w_in_sb,
                in_=w_in_c.rearrange("(kt p) m -> p kt m", p=128),
            )
            nc.sync.dma_start(out=w_out_sb, in_=w_out_c[:, :])
            if not USE_DMA_TRANSPOSE:
                ident = big.tile([128, 128], BF16)
                make_identity(nc, ident)
            nc.vector.memset(Vt[:, :, 64], 1.0)
            nc.vector.memset(Vt[:, :, 129], 1.0)

            xin_tiles = {}

            def emit_xin_dma(nb):
                xin = xinp.tile(
                    [128, KT_TILES, 2, 512], BF16, tag="xin", name=f"xin{nb}"
                )
                ncol = slice(nb * 1024, (nb + 1) * 1024)
                src = xT_r[:, :, ncol].rearrange("p kt (a b) -> p kt a b", b=512)
                for k in range(KT_TILES):
                    for a in range(2):
                        nc.sync.dma_start(
                            out=xin[:, k, a, :], in_=src[:, k, a, :]
                        )
                xin_tiles[nb] = xin

            # ---------- filler unit generators ----------
            def qkv_unit(nb, m, a):
                """8-chain projection matmul unit -> QT/KT/Vt columns."""
                pj = fillp.tile([128, 512], F32, tag="fill", name=f"pj{nb}{m}{a}")
                xin = xin_tiles[nb]
                for k in range(KT_TILES):
                    nc.tensor.matmul(
                        pj,
                        w_in_sb[:, k, m * 128:(m + 1) * 128],
                        xin[:, k, a, :],
                        start=(k == 0),
                        stop=(k == KT_TILES - 1),
                    )
                    yield 1
                cols = slice(nb * 1024 + a * 512, nb * 1024 + (a + 1) * 512)
                if m == 0:
                    nc.vector.tensor_copy(QT[:, cols], pj)
                elif m == 1:
                    nc.vector.tensor_copy(KT[:, cols], pj)
                else:
                    vstage = strm.tile([128, 512], BF16, tag="vstage")
                    nc.vector.tensor_copy(vstage, pj)
                    g0 = nb * 8 + a * 4
                    if USE_DMA_TRANSPOSE:
                        nc.sync.dma_start_transpose(
                            out=Vt[:, g0:g0 + 4, 0:64], in_=vstage[0:64, :]
                        )
                        nc.sync.dma_start_transpose(
                            out=Vt[:, g0:g0 + 4, 65:129], in_=vstage[64:128, :]
                        )
                        yield 0
                    else:
                        for c in range(4):
                            tp = fillp.tile(
                                [128, 128], BF16, tag="fill", name=f"tp{g0 + c}"
                            )
                            nc.tensor.transpose(
                                tp, vstage[:, c * 128:(c + 1) * 128], ident
                            )
                            # transpose LDWEIGHTS (~176ns) exceeds its compute
                            # (~92ns): cost 3 ends the pop call so transposes
                            # always have 512-wide matmuls between them
                            yield 3
                            nc.vector.tensor_copy(Vt[:, g0 + c, 0:64], tp[:, 0:64])
                            nc.vector.tensor_copy(
                                Vt[:, g0 + c, 65:129], tp[:, 64:128]
                            )
                        yield 0

            def proj_unit(bp, ihp, mt, a):
                """One output-projection matmul -> po DRAM (bf16 partial)."""
                i0 = bp * N + ihp * 1024 + a * 512
                pp = fillp.tile(
                    [128, 512], F32, tag="fill", name=f"pp{bp}{ihp}{mt}{a}"
                )
                nc.tensor.matmul(
                    pp,
                    w_out_sb[:, mt * 128:(mt + 1) * 128],
                    o_sb[:, i0:i0 + 512],
                    start=True,
                    stop=True,
                )
                yield 1
                pb = poutp.tile([128, 512], BF16, tag="pout")
                nc.vector.tensor_copy(pb, pp)
                nc.sync.dma_start(
                    out=po[mt * 128:(mt + 1) * 128, i0:i0 + 512], in_=pb
                )
                yield 0

            def dma_unit(fn, *args):
                fn(*args)
                yield 0

            class Filler:
                def __init__(self):
                    self.q = []
                    self.ndummy = 0

                def add(self, gen):
                    self.q.append(gen)

                def pop(self, n, dummy_ok=True):
                    got = 0
                    while got < n:
                        if not self.q:
                            if not dummy_ok:
                                return got
                            # dummy matmul: keeps the PE p-state ramped when
                            # real filler is exhausted; result never read
                            dj = fillp.tile(
                                [128, 512], F32, tag="fill",
                                name=f"dj{self.ndummy}",
                            )
                            self.ndummy += 1
                            nc.tensor.matmul(
                                dj, w_out_sb[:, 0:128], w_out_sb[:, 0:512],
                                start=True, stop=True,
                            )
                            got += 1
                            continue
                        try:
                            got += next(self.q[0])
                        except StopIteration:
                            self.q.pop(0)
                    return got

            filler = Filler()

            # ---------- attention chunk machinery ----------
            chunks = [
                (b, ih, h)
                for b in range(B)
                for ih in range(2)
                for h in range(HPC)
            ]

            def emit_scores(b, ih, h, jt, ci):
                hp = slice(h * 64, (h + 1) * 64)
                i0 = b * N + ih * 1024
                jcol = slice(b * N + jt * 128, b * N + jt * 128 + 128)
                st = stp.tile(
                    [128, 2, 512], F32, tag="st", name=f"st{ci}_{jt}"
                )
                for a in range(2):
                    nc.tensor.matmul(
                        st[:, a, :],
                        KT[hp, jcol],
                        QT[hp, i0 + a * 512:i0 + (a + 1) * 512],
                        start=True,
                        stop=True,
                    )
                et = etp.tile([128, 1024], BF16, tag="et", name="et")
                nc.scalar.activation(
                    et,
                    st.rearrange("p a b -> p (a b)"),
                    mybir.ActivationFunctionType.Exp,
                    scale=SCALE,
                )
                return et

            def emit_attnv(b, h, jp, et, po_t):
                for a in range(2):
                    nc.tensor.matmul(
                        po_t[:, a, :],
                        Vt[:, b * JT + jp, h * 65:h * 65 + 65],
                        et[:, a * 512:(a + 1) * 512],
                        start=(jp == 0),
                        stop=(jp == JT - 1),
                    )

            def emit_drain(b, ih, h, po_t):
                # one copy frees the po psum bank fast; o rows and the
                # (bf16) denominator row ship out via DMA afterwards
                i0 = b * N + ih * 1024
                icol = slice(i0, i0 + 1024)
                seg = b * 4 + ih * 2 + h
                stage = strm.tile([65, 1024], BF16, tag="stage")
                nc.vector.tensor_copy(
                    stage, po_t.rearrange("p a b -> p (a b)")
                )
                rows = slice(h * 64, (h + 1) * 64)
                nc.sync.dma_start(out=o_sb[rows, icol], in_=stage[0:64, :])
                nc.sync.dma_start(
                    out=dn_dram[seg:seg + 1, :], in_=stage[64:65, :]
                )

            def emit_normalize(b, ih, hh):
                # per-head normalize, emitted one chunk after the drain so the
                # dn DRAM round trip completes before the DVE touches it
                i0 = b * N + ih * 1024
                icol = slice(i0, i0 + 1024)
                seg = b * 4 + ih * 2 + hh
                dns = strm.tile([128, 8], BF16, tag="dns")
                nc.sync.dma_start(
                    out=dns,
                    in_=dn_dram[seg:seg + 1, :].rearrange(
                        "o (p a) -> (o p) a", p=128
                    ),
                )
                rcc = strm.tile([128, 8], BF16, tag="rcc")
                with nc.allow_low_precision(reason="softmax denom recip"):
                    nc.vector.reciprocal(rcc, dns)
                nc.sync.dma_start(
                    out=rc_dram[seg:seg + 1, :].rearrange(
                        "o (p a) -> (o p) a", p=128
                    ),
                    in_=rcc,
                )
                rows = slice(hh * 64, (hh + 1) * 64)
                bcast = strm.tile([128, 1024], BF16, tag="bcast")
                src = rc_dram[seg:seg + 1, :]
                rbc = bass.AP(
                    tensor=src.tensor,
                    offset=src.offset,
                    ap=[[0, 64]] + list(src.ap)[1:],
                )
                nc.sync.dma_start(out=bcast[rows, :], in_=rbc)
                nc.vector.tensor_mul(
                    o_sb[rows, icol], o_sb[rows, icol], bcast[rows, :]
                )

            # ---------- build filler queue: prologue qkv for b0 ----------
            emit_xin_dma(0)
            emit_xin_dma(1)
            for nb in range(2):
                for m in range(3):
                    for a in range(2):
                        filler.add(qkv_unit(nb, m, a))
            filler.pop(10 ** 6, dummy_ok=False)  # prologue: drain all of b0

            # queue qkv for batches 1..3 (consumed as filler during
            # attention); keep xin DMAs 2 blocks ahead of their consumers
            emit_xin_dma(2)
            emit_xin_dma(3)
            for nb in range(2, NNB):
                if nb + 2 < NNB:
                    filler.add(dma_unit(emit_xin_dma, nb + 2))
                for m in range(3):
                    for a in range(2):
                        filler.add(qkv_unit(nb, m, a))

            # ---------- main loop: one global software pipeline ----------
            # slot g: emit scores(g+2) then attnV(g); the scores stream runs
            # 2 slots ahead ACROSS chunk boundaries so the scalar exp queue
            # never drains and the st-buffer wall is never hit
            pending_norm = []
            slots = [(ci, jt) for ci in range(len(chunks)) for jt in range(JT)]
            S = len(slots)
            ets = {}
            po_ts = {}

            def scores_side(g):
                ci, jt = slots[g]
                b, ih, h = chunks[ci]
                ets[g] = emit_scores(b, ih, h, jt, ci)

            def attnv_side(g):
                ci, jt = slots[g]
                b, ih, h = chunks[ci]
                if jt == 0:
                    po_ts[ci] = popp.tile(
                        [65, 2, 512], F32, tag="po", name=f"po{ci}"
                    )
                emit_attnv(b, h, jt, ets.pop(g), po_ts[ci])
                if jt == JT - 1:
                    emit_drain(b, ih, h, po_ts.pop(ci))
                    pending_norm.append((b, ih, h))

            scores_side(0)
            scores_side(1)
            acc = 2.0
            rate = 2.0
            for g in range(S):
                if g + 2 < S:
                    scores_side(g + 2)
                attnv_side(g)
                ci, jt = slots[g]
                if jt == 2:
                    while pending_norm:
                        nb_, nih_, nh_ = pending_norm.pop(0)
                        emit_normalize(nb_, nih_, nh_)
                        if nh_ == 1:
                            for mt in range(DIM // 128):
                                for a in range(2):
                                    filler.add(proj_unit(nb_, nih_, mt, a))
                # extra pops at chunk tails cover the drain copy latency
                # before the next chunk's attnV(jt=0) needs the po bank
                acc += rate + (3.0 if jt == JT - 1 else 0.0)
                npop = int(acc)
                if npop:
                    filler.pop(npop)
                    acc -= npop

            # epilogue: last chunk's normalize + projection; dummy matmuls
            # keep the PE ramped while the dn/rc DMA round trips complete
            filler.pop(10)
            while pending_norm:
                nb_, nih_, nh_ = pending_norm.pop(0)
                emit_normalize(nb_, nih_, nh_)
                if nh_ == 1:
                    for mt in range(DIM // 128):
                        for a in range(2):
                            filler.add(proj_unit(nb_, nih_, mt, a))
            filler.pop(25)
            filler.pop(10 ** 6, dummy_ok=False)

    nc.finalize()
    return nc


_CACHED = {}


def kernel(x, w_in, w_out, b_out, _trace=False):
    if "nc" not in _CACHED:
        _CACHED["nc"] = _build_nc()
    nc = _CACHED["nc"]

    x2 = np.ascontiguousarray(
        x.reshape(NT, DIM).T.astype(np.float32)
    )  # [DIM, NT]
    in_maps = []
    for c in range(NCORES):
        h0, h1 = HPC * c, HPC * c + 1
        cols = []
        for part in range(3):  # q, k, v
            base = part * DIM
            cols.extend(range(base + h0 * HD, base + h0 * HD + HD))
            cols.extend(range(base + h1 * HD, base + h1 * HD + HD))
        w_in_cc = np.ascontiguousarray(w_in[:, cols].astype(np.float32))
        w_out_cc = np.ascontiguousarray(
            w_out[128 * c:128 * (c + 1), :].astype(np.float32)
        )
        in_maps.append(
            {
                "xT": x2.astype(ml_dtypes.bfloat16),
                "w_in_c": w_in_cc.astype(ml_dtypes.bfloat16),
                "w_out_c": w_out_cc.astype(ml_dtypes.bfloat16),
            }
        )

    res = run_bass_kernel_spmd(
        nc, in_maps, core_ids=list(range(NCORES)), trace=_trace
    )
    acc = res.results[0]["po"].astype(np.float64)
    for c in range(1, NCORES):
        acc = acc + res.results[c]["po"].astype(np.float64)
    out = acc.T + b_out.astype(np.float64)
    if _trace:
        kernel.last_result = res
    return np.ascontiguousarray(out.reshape(B, N, DIM).astype(np.float32))
